# revision 29
# baseline (speedup 1.0000x reference)
"""AttentionWithRoPE Trainium2 kernel (8-core SPMD), v2.

Sharding: core c handles batch b = c // 2 and head-group g = c % 2
(heads 4g..4g+3).  Each core computes rmsnorm(x_b), its 4 heads' Q/K/V
projections, RoPE, full-sequence attention, and a partial output
projection (its heads' rows of w_out).  Host sums the two partial
outputs per batch.

v2 changes vs v1:
- fp16 everywhere on the PE (fp32-HIGH mode and its FWL-disable penalty
  are gone); accumulation stays fp32 in PSUM.
- RoPE pair-swap is a single DVE stream_shuffle: head dims are
  host-permuted so swap partners sit +-16 apart inside each 32-partition
  quadrant (stream_shuffle can only permute within quadrants).
- Attention runs one head at a time, software-pipelined: per k-tile the
  PE emits logits, ACT exps them (1024 wide), and the PE retires the
  previous k-tile's attn@V while the exp runs.  ACT (exp) is the
  critical engine; the PE stream is kept dense so the HAM clock stays
  at 2.4 GHz.
- Long-latency work (V projection, later RoPE chunks, output projection
  tiles) is emitted as "fillers" inside attention kt loops so no engine
  idles between phases.
- Softmax denominator comes from a ones column appended to V; the
  normalize (broadcast/recip/mul) is deferred off the critical path via
  an SBUF copy of the raw attention output.
- PSUM budget (8 banks): work pool 2, logits double-buffer 4, attn-out 2.
"""

import os

import numpy as np

import concourse.bass as bass
import concourse.tile as tile
from concourse import bacc, mybir

B, N, DIM = 4, 2048, 512
H, D = 8, 64
ROPE_THETA = 10000.0
NCORES = 8
SCALE = D ** -0.5

F32 = mybir.dt.float32
F16 = mybir.dt.float16

SWAP16 = [(i + 16) % 32 for i in range(32)]


def build_program():
    nc = bacc.Bacc("TRN2", target_bir_lowering=False, debug=False)

    xT = nc.dram_tensor("xT", [DIM, N], F16, kind="ExternalInput").ap()
    wqk = nc.dram_tensor("wqk", [DIM, 512], F16, kind="ExternalInput").ap()
    wv = nc.dram_tensor("wv", [DIM, 256], F16, kind="ExternalInput").ap()
    wo = nc.dram_tensor("wo", [256, DIM], F16, kind="ExternalInput").ap()
    cos2 = nc.dram_tensor("cos2", [128, N], F16, kind="ExternalInput").ap()
    sinF2 = nc.dram_tensor("sinF2", [128, N], F16, kind="ExternalInput").ap()
    ones_d = nc.dram_tensor("ones", [128, 128], F16, kind="ExternalInput").ap()
    yT = nc.dram_tensor("yT", [DIM, N], F32, kind="ExternalOutput").ap()
    debug = bool(os.environ.get("KERNEL_DEBUG"))
    if debug:
        dbg_qr = nc.dram_tensor("dbg_qr", [128, N], F16, kind="ExternalOutput").ap()
        dbg_kr = nc.dram_tensor("dbg_kr", [128, N], F16, kind="ExternalOutput").ap()
        dbg_e = nc.dram_tensor("dbg_e", [128, 1024], F16, kind="ExternalOutput").ap()
        dbg_oraw = nc.dram_tensor("dbg_oraw", [65, 1024], F32, kind="ExternalOutput").ap()
        dbg_outd = nc.dram_tensor("dbg_outd", [128, 1024], F16, kind="ExternalOutput").ap()
        dbg_v = nc.dram_tensor("dbg_v", [128, 260], F16, kind="ExternalOutput").ap()
        dbg_sinv = nc.dram_tensor("dbg_sinv", [128, N], F32, kind="ExternalOutput").ap()

    with tile.TileContext(nc) as tc:
        with tc.tile_pool(name="persist", bufs=1) as persist, \
             tc.tile_pool(name="xsqp", bufs=2) as xsqp, \
             tc.tile_pool(name="ropework", bufs=3) as ropework, \
             tc.tile_pool(name="exps", bufs=6) as exps, \
             tc.tile_pool(name="orawp", bufs=3) as orawp, \
             tc.tile_pool(name="normp", bufs=2) as normp, \
             tc.tile_pool(name="ysbp", bufs=2) as ysbp, \
             tc.tile_pool(name="ps_w", bufs=2, space="PSUM") as ps_w, \
             tc.tile_pool(name="ps_s", bufs=2, space="PSUM") as ps_s, \
             tc.tile_pool(name="ps_o", bufs=1, space="PSUM") as ps_o:

            # ---- input DMAs, column-chunked so the rmsnorm pipeline can
            # start on the first 512 tokens while the rest stream in ----
            ones128 = persist.tile([128, 128], F16, tag="ones128", name="ones128")
            nc.sync.dma_start(ones128[:], ones_d)
            wqk_t = []
            for i in range(4):
                t = persist.tile([128, 512], F16, tag=f"wqk{i}", name=f"wqk{i}")
                nc.sync.dma_start(t[:], wqk[i * 128:(i + 1) * 128, :])
                wqk_t.append(t)
            xt = [persist.tile([128, N], F16, tag=f"xt{i}", name=f"xt_{i}",
                               uniquify=False) for i in range(4)]
            for c in range(4):
                cs = slice(c * 512, (c + 1) * 512)
                for i in range(4):
                    nc.sync.dma_start(xt[i][:, cs], xT[i * 128:(i + 1) * 128, cs])
                if c == 0:
                    cos_t = persist.tile([128, N], F16, tag="cos", name="cos")
                    nc.sync.dma_start(cos_t[:], cos2)
                    sin_t = persist.tile([128, N], F16, tag="sin", name="sin")
                    nc.sync.dma_start(sin_t[:], sinF2)
            wv_t = []
            for i in range(4):
                t = persist.tile([128, 256], F16, tag=f"wv{i}", name=f"wv{i}")
                nc.sync.dma_start(t[:], wv[i * 128:(i + 1) * 128, :])
                wv_t.append(t)
            wo_t = []
            for p in range(2):
                t = persist.tile([128, 512], F16, tag=f"wo{p}", name=f"wo{p}")
                nc.sync.dma_start(t[:], wo[p * 128:(p + 1) * 128, :])
                wo_t.append(t)
            # V tiles: memset to 1.0; the V-copy overwrites the data columns,
            # leaving the per-head ones column (softmax denominator) intact.
            v_sb = []
            for tt in range(16):
                t = persist.tile([128, 260], F16, tag=f"v{tt}", name=f"v{tt}")
                nc.gpsimd.memset(t[:], 1.0)
                v_sb.append(t)

            # ---- rmsnorm, per 512-token chunk: sumsq (ones-matmul on
            # gpsimd-squared x), sqrt, recip, xn ----
            ss_ab = [ps_s.tile([128, 1024], F32, tag="s", name=f"ss{a}")
                     for a in range(2)]
            sn = persist.tile([128, N], F32, tag="sn", name="sn")
            sinv = persist.tile([128, N], F32, tag="sinv", name="sinv")
            xn = [persist.tile([128, N], F16, tag=f"xn{i}", name=f"xn_{i}",
                               uniquify=False) for i in range(4)]
            for c in range(4):
                cs = slice(c * 512, (c + 1) * 512)
                hs = slice((c % 2) * 512, (c % 2) * 512 + 512)
                for i in range(4):
                    xsq = xsqp.tile([128, 512], F16, tag="xsq", name="xsq")
                    # first chunk squares on DVE (shorter critical path to
                    # the first exp); the rest on the otherwise-idle gpsimd
                    if c == 0:
                        nc.vector.tensor_mul(xsq[:], xt[i][:, cs],
                                             xt[i][:, cs])
                    else:
                        nc.gpsimd.tensor_mul(xsq[:], xt[i][:, cs],
                                             xt[i][:, cs])
                    nc.tensor.matmul(ss_ab[c // 2][:, hs], ones128[:],
                                     xsq[:], start=(i == 0), stop=(i == 3),
                                     skip_group_check=True)
                # sn = sqrt(sumsq/512)  ->  sinv = sqrt(512)/||x||
                nc.scalar.activation(sn[:, cs], ss_ab[c // 2][:, hs],
                                     mybir.ActivationFunctionType.Sqrt,
                                     scale=1.0 / DIM)
                nc.vector.reciprocal_approx_fast(sinv[:, cs], sn[:, cs])
                for i in range(4):
                    nc.vector.tensor_mul(xn[i][:, cs], xt[i][:, cs],
                                         sinv[:, cs])

            # ---- Q/K projection + RoPE ----
            # wqk columns: [q h0..h3 | k h0..h3]; m=0: q heads01, m=1: q
            # heads23, m=2: k heads01, m=3: k heads23.  Head d-dims are
            # host-permuted so the RoPE pair-swap is partition p <-> p^16
            # within each 32-partition quadrant (one stream_shuffle).
            qk_dest = []
            for name in ["qr0", "qr1", "kr0", "kr1"]:
                t = persist.tile([128, N], F16, tag=name, name=name)
                qk_dest.append(t)

            def emit_rope_chunk(m, c):
                ms = slice(m * 128, (m + 1) * 128)
                cs = slice(c * 512, (c + 1) * 512)
                qk = ps_w.tile([128, 512], F32, tag="w", name="qkps")
                for i in range(4):
                    nc.tensor.matmul(qk[:], wqk_t[i][:, ms], xn[i][:, cs],
                                     start=(i == 0), stop=(i == 3))
                qkraw = ropework.tile([128, 512], F16, tag="qkraw", name="qkraw")
                nc.vector.tensor_copy(qkraw[:], qk[:])
                rotu = ropework.tile([128, 512], F16, tag="rotu", name="rotu")
                nc.vector.stream_shuffle(rotu[:].bitcast(mybir.dt.int32),
                                         qkraw[:].bitcast(mybir.dt.int32),
                                         SWAP16)
                tmpc = ropework.tile([128, 512], F16, tag="tmpc", name="tmpc")
                nc.vector.tensor_mul(tmpc[:], qkraw[:], cos_t[:, cs])
                rots = ropework.tile([128, 512], F16, tag="rots", name="rots")
                nc.vector.tensor_mul(rots[:], rotu[:], sin_t[:, cs])
                nc.vector.tensor_add(qk_dest[m][:, cs], tmpc[:], rots[:])

            # ---- V projection (token-major), via filler units ----
            def emit_v(tt):
                vp = ps_w.tile([128, 512], F32, tag="w", name="vps")
                ts = slice(tt * 128, (tt + 1) * 128)
                for i in range(4):
                    nc.tensor.matmul(vp[:, 0:256], xn[i][:, ts], wv_t[i][:],
                                     start=(i == 0), stop=(i == 3))
                # cols 65h..65h+63 hold head h's V; col 65h+64 stays 1.0
                # (softmax denominator lands in o_ps row 64).
                dst = v_sb[tt][:].rearrange("p (h c) -> p h c", h=4)[:, :, 0:64]
                nc.vector.tensor_copy(dst, vp[:, 0:256].rearrange(
                    "p (h c) -> p h c", h=4))

            # ---- attention for one (query-half, head) ----
            # Software-pipelined: attn@V trails exp by 2 k-tiles, and the
            # final two attn@V tiles + normalize of head h are carried into
            # head h+1's loop so the ACT exp stream never waits.
            outd = [[None, None], [None, None]]  # [hp][qh] -> [128,1024] f16

            def emit_attention(qh, h, fillers, carry, last=False):
                hp, j = h // 2, h % 2
                qr, kr = qk_dest[hp], qk_dest[2 + hp]
                js = slice(j * 64, (j + 1) * 64)
                o_ps = ps_o.tile([65, 1024], F32, tag="o", name="o")
                lag = 1 if last else 2
                es = []
                for kt in range(16):
                    if fillers:
                        fillers.pop(0)()
                    ks = slice(kt * 128, (kt + 1) * 128)
                    s_ps = ps_s.tile([128, 1024], F32, tag="s", name="sc")
                    for sub in range(2):
                        qs = slice(qh * 1024 + sub * 512,
                                   qh * 1024 + (sub + 1) * 512)
                        nc.tensor.matmul(s_ps[:, sub * 512:(sub + 1) * 512],
                                         kr[js, ks], qr[js, qs],
                                         start=True, stop=True)
                    e = exps.tile([128, 1024], F16, tag="e", name="e")
                    nc.scalar.activation(e[:], s_ps[:],
                                         mybir.ActivationFunctionType.Exp,
                                         scale=SCALE)
                    if debug and qh == 0 and h == 0 and kt == 0:
                        nc.sync.dma_start(dbg_e, e[:])
                    es.append(e)
                    if kt == 0 and carry:
                        carry[0]()
                    elif kt == 1 and carry:
                        carry[1]()
                    if kt >= lag:
                        _av(o_ps, h, kt - lag, es[kt - lag], kt == lag, False)
                if last:
                    # tail: finish attn@V and normalize straight from PSUM
                    _av(o_ps, h, 15, es[15], False, True)
                    dcopy = normp.tile([1, 1024], F32, tag="dcopy",
                                       name="dcopy")
                    nc.vector.tensor_copy(dcopy[:], o_ps[64:65, :])
                    if outd[hp][qh] is None:
                        outd[hp][qh] = persist.tile(
                            [128, 1024], F16, tag=f"od{hp}{qh}",
                            name=f"od{hp}{qh}_l")
                    rrow = normp.tile([1, 1024], F32, tag="rrow", name="rrow")
                    nc.vector.reciprocal_approx_fast(rrow[:], dcopy[:])
                    rfull = normp.tile([64, 1024], F32, tag="rfull",
                                       name="rfull")
                    nc.gpsimd.partition_broadcast(rfull[:], rrow[:])
                    nc.vector.tensor_mul(outd[hp][qh][js, :], o_ps[0:64, :],
                                         rfull[:])
                    return []

                def carry_av():
                    _av(o_ps, h, 14, es[14], False, False)
                    _av(o_ps, h, 15, es[15], False, True)

                def carry_norm():
                    oraw = orawp.tile([64, 1024], F32, tag="oraw", name="oraw")
                    nc.vector.tensor_copy(oraw[:], o_ps[0:64, :])
                    dcopy = normp.tile([1, 1024], F32, tag="dcopy",
                                       name="dcopy")
                    nc.vector.tensor_copy(dcopy[:], o_ps[64:65, :])
                    if debug and qh == 0 and h == 0:
                        nc.sync.dma_start(dbg_oraw[0:64, :], oraw[:])
                        nc.sync.dma_start(dbg_oraw[64:65, :], dcopy[:])
                    if outd[hp][qh] is None:
                        od = persist.tile([128, 1024], F16, tag=f"od{hp}{qh}",
                                          name=f"od{hp}{qh}")
                        outd[hp][qh] = od
                    # gpsimd/custom-DVE ucode mishandles nonzero partition
                    # offsets on HW: extract the denominator row with a
                    # plain copy so recip/broadcast see offset-0 APs.
                    rrow = normp.tile([1, 1024], F32, tag="rrow", name="rrow")
                    nc.vector.reciprocal_approx_fast(rrow[:], dcopy[:])
                    rfull = normp.tile([64, 1024], F32, tag="rfull",
                                       name="rfull")
                    nc.gpsimd.partition_broadcast(rfull[:], rrow[:])
                    nc.vector.tensor_mul(outd[hp][qh][js, :], oraw[:],
                                         rfull[:])

                return [carry_av, carry_norm]

            def _av(o_ps, h, kt, e, start, stop):
                for sub in range(2):
                    ss_ = slice(sub * 512, (sub + 1) * 512)
                    nc.tensor.matmul(o_ps[:, ss_],
                                     v_sb[kt][:, 65 * h:65 * h + 65],
                                     e[:, ss_], start=start, stop=stop,
                                     skip_group_check=True)

            # ---- output projection for one query-half (partial w_out) ----
            def emit_proj_chunk(qh, om, sub, copy_eng="vector", dma_each=False):
                oms = slice(om * 128, (om + 1) * 128)
                ss_ = slice(sub * 512, (sub + 1) * 512)
                yp = ps_w.tile([128, 512], F32, tag="w", name="yp")
                for p in range(2):
                    nc.tensor.matmul(yp[:], wo_t[p][:, oms],
                                     outd[p][qh][:, ss_],
                                     start=(p == 0), stop=(p == 1))
                if sub == 0:
                    _ylast[0] = ysbp.tile([128, 1024], F32, tag="y", name="y")
                ysb = _ylast[0]
                if copy_eng == "vector":
                    nc.vector.tensor_copy(ysb[:, ss_], yp[:])
                else:
                    nc.scalar.copy(ysb[:, ss_], yp[:])
                if dma_each:
                    nc.sync.dma_start(
                        yT[oms, qh * 1024 + sub * 512:
                           qh * 1024 + (sub + 1) * 512], ysb[:, ss_])
                elif sub == 1:
                    nc.sync.dma_start(
                        yT[oms, qh * 1024:(qh + 1) * 1024], ysb[:])

            _ylast = [None]

            # ---- emission order ----
            # Minimal prefix so attention(qh0, h0) starts as early as
            # possible: kr0/qr0 first 512-token chunks plus the first few V
            # tiles (their PE work overlaps the serial rope DVE chain).
            emit_rope_chunk(2, 0)              # kr0 c0
            emit_rope_chunk(0, 0)              # qr0 c0
            emit_rope_chunk(0, 1)              # qr0 c1
            for tt in range(6):
                emit_v(tt)
            emit_rope_chunk(2, 1)              # kr0 c1..c3
            emit_rope_chunk(2, 2)
            emit_rope_chunk(2, 3)

            fillers = []
            for tt in range(6, 16):
                fillers.append(lambda tt=tt: emit_v(tt))
            for c in range(4):
                fillers.append(lambda c=c: emit_rope_chunk(3, c))   # kr1
            for c in range(2):
                fillers.append(lambda c=c: emit_rope_chunk(1, c))   # qr1 qh0
            for c in range(2, 4):
                fillers.append(lambda c=c: emit_rope_chunk(0, c))   # qr0 qh1
            for c in range(2, 4):
                fillers.append(lambda c=c: emit_rope_chunk(1, c))   # qr1 qh1

            carry = []
            for h in range(4):
                carry = emit_attention(0, h, fillers, carry)
            if debug:
                nc.sync.dma_start(dbg_qr, qk_dest[0][:])
                nc.sync.dma_start(dbg_kr, qk_dest[2][:])
                nc.sync.dma_start(dbg_v, v_sb[0][:])
                nc.sync.dma_start(dbg_sinv, sinv[:])
            # outproj(qh0) becomes fillers, but only after att(1,0) has
            # consumed the carried normalize of (qh0, h3): delay them into
            # the att(1,1)+ loops to keep the DVE stream deadlock-free.
            carry = emit_attention(1, 0, fillers, carry)
            for om in range(4):
                for sub in range(2):
                    fillers.append(
                        lambda om=om, sub=sub: emit_proj_chunk(0, om, sub))
            for h in range(1, 4):
                carry = emit_attention(1, h, fillers, carry, last=(h == 3))
            assert not fillers and not carry
            if debug:
                nc.sync.dma_start(dbg_outd, outd[0][0][:])
            for om in range(4):
                for sub in range(2):
                    emit_proj_chunk(1, om, sub,
                                    copy_eng="scalar" if om % 2 else "vector",
                                    dma_each=True)

    nc.compile()
    return nc


# Device row r (within a 64-row head slot) holds head-dim PERM64[r]:
# quadrant-local 16-interleave so the RoPE pair partner is at r^16.
PERM64 = np.array([2 * (16 * (r // 32) + r % 16) + ((r % 32) // 16)
                   for r in range(D)])
_SUB = np.array([(r % 32) // 16 for r in range(D)])
_J = np.array([16 * (r // 32) + r % 16 for r in range(D)])


def rope_tables():
    """cos / sign-folded sin tables in PERM64 row order, two head slots."""
    inv_freq = (1.0 / (ROPE_THETA ** (np.arange(0, D, 2, dtype=np.float32) / D)))
    freqs = np.arange(N, dtype=np.float32)[:, None] * inv_freq[None, :]  # [N,32]
    cos64 = np.cos(freqs[:, _J]).T.astype(np.float32)   # [64, N]
    sin64 = np.sin(freqs[:, _J]).T.astype(np.float32)
    sinF64 = np.where(_SUB[:, None] == 0, -sin64, sin64)
    cos2 = np.concatenate([cos64, cos64], axis=0)        # [128, N]
    sinF2 = np.concatenate([sinF64, sinF64], axis=0)
    return (np.ascontiguousarray(cos2).astype(np.float16),
            np.ascontiguousarray(sinF2).astype(np.float16))


def _permute_heads(w):
    """Permute each head's 64 columns of w [512, 256] to PERM64 order."""
    w = w.reshape(DIM, 4, D)[:, :, PERM64]
    return w.reshape(DIM, 256)


def make_in_maps(x, gamma, w_qkv, w_out):
    cos2, sinF2 = rope_tables()
    wg = (gamma[:, None] * w_qkv).astype(np.float32)  # fold gamma
    in_maps = []
    for c in range(NCORES):
        b, g = c // 2, c % 2
        hs = slice(g * 256, (g + 1) * 256)
        wqk_c = np.concatenate([_permute_heads(wg[:, 0:512][:, hs]),
                                _permute_heads(wg[:, 512:1024][:, hs])],
                               axis=1)
        wv_c = wg[:, 1024:1536][:, hs]
        wo_c = w_out[hs, :]
        in_maps.append({
            "xT": np.ascontiguousarray(x[b].T).astype(np.float16),
            "wqk": np.ascontiguousarray(wqk_c).astype(np.float16),
            "wv": np.ascontiguousarray(wv_c).astype(np.float16),
            "wo": np.ascontiguousarray(wo_c).astype(np.float16),
            "cos2": cos2,
            "sinF2": sinF2,
            "ones": np.ones((128, 128), dtype=np.float16),
        })
    return in_maps


_NC_CACHE = None


def _get_program():
    global _NC_CACHE
    if _NC_CACHE is None:
        _NC_CACHE = build_program()
    return _NC_CACHE


def run_cores(inputs, trace=False):
    """Run the SPMD kernel on 8 cores; returns (full_output, results)."""
    from concourse.bass_utils import run_bass_kernel_spmd

    nc = _get_program()
    in_maps = make_in_maps(inputs["x"], inputs["gamma"],
                           inputs["w_qkv"], inputs["w_out"])
    kwargs = {}
    if trace:
        _install_ntff_hook()
        kwargs = dict(trace=True, trace_cores=list(range(NCORES)))
    res = run_bass_kernel_spmd(nc, in_maps, core_ids=list(range(NCORES)),
                               **kwargs)
    out = np.empty((B, N, DIM), dtype=np.float32)
    for b in range(B):
        yTv = res.results[2 * b]["yT"] + res.results[2 * b + 1]["yT"]
        out[b] = yTv.T
    return out, res


def _install_ntff_hook():
    """Register the axon NTFF profiling hook (missing antenv.axon_hooks)."""
    import sys
    import types

    if "antenv.axon_hooks" in sys.modules:
        return
    try:
        import trn_agent_boot.trn_boot as tb
        import concourse.bass_utils as bu

        mod = types.ModuleType("antenv.axon_hooks")
        hook = tb._ntff_profile_via_ctypes("/opt/axon/libaxon_pjrt.so")
        mod.get_axon_ntff_profile_hook = lambda: hook
        sys.modules["antenv.axon_hooks"] = mod
        bu.upload_artifacts = lambda tmpdir: "local://" + tmpdir
    except Exception:
        pass


def kernel(**inputs):
    out, _ = run_cores(inputs, trace=bool(os.environ.get("KERNEL_TRACE")))
    return out


# revision 36
# speedup vs baseline: 1.0163x; 1.0163x over previous
"""AttentionWithRoPE Trainium2 kernel (8-core SPMD), v2.

Sharding: core c handles batch b = c // 2 and head-group g = c % 2
(heads 4g..4g+3).  Each core computes rmsnorm(x_b), its 4 heads' Q/K/V
projections, RoPE, full-sequence attention, and a partial output
projection (its heads' rows of w_out).  Host sums the two partial
outputs per batch.

v2 changes vs v1:
- fp16 everywhere on the PE (fp32-HIGH mode and its FWL-disable penalty
  are gone); accumulation stays fp32 in PSUM.
- RoPE pair-swap is a single DVE stream_shuffle: head dims are
  host-permuted so swap partners sit +-16 apart inside each 32-partition
  quadrant (stream_shuffle can only permute within quadrants).
- Attention runs one head at a time, software-pipelined: per k-tile the
  PE emits logits, ACT exps them (1024 wide), and the PE retires the
  previous k-tile's attn@V while the exp runs.  ACT (exp) is the
  critical engine; the PE stream is kept dense so the HAM clock stays
  at 2.4 GHz.
- Long-latency work (V projection, later RoPE chunks, output projection
  tiles) is emitted as "fillers" inside attention kt loops so no engine
  idles between phases.
- Softmax denominator comes from a ones column appended to V; the
  normalize (broadcast/recip/mul) is deferred off the critical path via
  an SBUF copy of the raw attention output.
- PSUM budget (8 banks): work pool 2, logits double-buffer 4, attn-out 2.
"""

import os

import numpy as np

import concourse.bass as bass
import concourse.tile as tile
from concourse import bacc, mybir

B, N, DIM = 4, 2048, 512
H, D = 8, 64
ROPE_THETA = 10000.0
NCORES = 8
SCALE = D ** -0.5

F32 = mybir.dt.float32
F16 = mybir.dt.float16

SWAP16 = [(i + 16) % 32 for i in range(32)]


def build_program():
    nc = bacc.Bacc("TRN2", target_bir_lowering=False, debug=False)

    xT = nc.dram_tensor("xT", [DIM, N], F16, kind="ExternalInput").ap()
    wqk = nc.dram_tensor("wqk", [DIM, 512], F16, kind="ExternalInput").ap()
    wv = nc.dram_tensor("wv", [DIM, 256], F16, kind="ExternalInput").ap()
    wo = nc.dram_tensor("wo", [256, DIM], F16, kind="ExternalInput").ap()
    cos2 = nc.dram_tensor("cos2", [128, N], F16, kind="ExternalInput").ap()
    sinF2 = nc.dram_tensor("sinF2", [128, N], F16, kind="ExternalInput").ap()
    ones_d = nc.dram_tensor("ones", [128, 128], F16, kind="ExternalInput").ap()
    yT = nc.dram_tensor("yT", [DIM, N], F32, kind="ExternalOutput").ap()
    debug = bool(os.environ.get("KERNEL_DEBUG"))
    if debug:
        dbg_qr = nc.dram_tensor("dbg_qr", [128, N], F16, kind="ExternalOutput").ap()
        dbg_kr = nc.dram_tensor("dbg_kr", [128, N], F16, kind="ExternalOutput").ap()
        dbg_e = nc.dram_tensor("dbg_e", [128, 1024], F16, kind="ExternalOutput").ap()
        dbg_oraw = nc.dram_tensor("dbg_oraw", [65, 1024], F32, kind="ExternalOutput").ap()
        dbg_outd = nc.dram_tensor("dbg_outd", [128, 1024], F16, kind="ExternalOutput").ap()
        dbg_v = nc.dram_tensor("dbg_v", [128, 260], F16, kind="ExternalOutput").ap()
        dbg_sinv = nc.dram_tensor("dbg_sinv", [128, N], F32, kind="ExternalOutput").ap()

    with tile.TileContext(nc) as tc:
        with tc.tile_pool(name="persist", bufs=1) as persist, \
             tc.tile_pool(name="xsqp", bufs=2) as xsqp, \
             tc.tile_pool(name="ropework", bufs=3) as ropework, \
             tc.tile_pool(name="exps", bufs=8) as exps, \
             tc.tile_pool(name="orawp", bufs=3) as orawp, \
             tc.tile_pool(name="normp", bufs=2) as normp, \
             tc.tile_pool(name="ysbp", bufs=2) as ysbp, \
             tc.tile_pool(name="ps_w", bufs=2, space="PSUM") as ps_w, \
             tc.tile_pool(name="ps_s", bufs=2, space="PSUM") as ps_s, \
             tc.tile_pool(name="ps_o", bufs=1, space="PSUM") as ps_o:

            # ---- input DMAs, ordered so the first 1024 tokens' rmsnorm
            # pipeline starts while the rest stream in ----
            xt = [persist.tile([128, N], F16, tag=f"xt{i}", name=f"xt_{i}",
                               uniquify=False) for i in range(4)]
            for c in range(2):
                cs = slice(c * 512, (c + 1) * 512)
                for i in range(4):
                    nc.sync.dma_start(xt[i][:, cs], xT[i * 128:(i + 1) * 128, cs])
            ones128 = persist.tile([128, 128], F16, tag="ones128", name="ones128")
            nc.sync.dma_start(ones128[:], ones_d)
            cos_t = persist.tile([128, N], F16, tag="cos", name="cos")
            nc.sync.dma_start(cos_t[:], cos2)
            sin_t = persist.tile([128, N], F16, tag="sin", name="sin")
            nc.sync.dma_start(sin_t[:], sinF2)
            for i in range(4):
                nc.sync.dma_start(xt[i][:, 1024:2048],
                                  xT[i * 128:(i + 1) * 128, 1024:2048])
            wqk_t = []
            for i in range(4):
                t = persist.tile([128, 512], F16, tag=f"wqk{i}", name=f"wqk{i}")
                nc.sync.dma_start(t[:], wqk[i * 128:(i + 1) * 128, :])
                wqk_t.append(t)
            wv_t = []
            for i in range(4):
                t = persist.tile([128, 256], F16, tag=f"wv{i}", name=f"wv{i}")
                nc.sync.dma_start(t[:], wv[i * 128:(i + 1) * 128, :])
                wv_t.append(t)
            wo_t = []
            for p in range(2):
                t = persist.tile([128, 512], F16, tag=f"wo{p}", name=f"wo{p}")
                nc.sync.dma_start(t[:], wo[p * 128:(p + 1) * 128, :])
                wo_t.append(t)
            # V tiles: memset to 1.0 (in emit_v); the V-copy overwrites the
            # data columns, leaving the per-head ones column (softmax
            # denominator) intact.
            v_sb = [persist.tile([128, 260], F16, tag=f"v{tt}", name=f"v_{tt}",
                                 uniquify=False) for tt in range(16)]

            # ---- rmsnorm, per 1024-token pair: sumsq via ones-matmul,
            # sqrt, recip, xn.  Pair 0 squares on DVE (critical path to the
            # first exp); pair 1 on the otherwise-idle gpsimd. ----
            ss_ab = [ps_s.tile([128, 1024], F32, tag="s", name=f"ss{a}")
                     for a in range(2)]
            sn = persist.tile([128, N], F32, tag="sn", name="sn")
            sinv = persist.tile([128, N], F32, tag="sinv", name="sinv")
            xn = [persist.tile([128, N], F16, tag=f"xn{i}", name=f"xn_{i}",
                               uniquify=False) for i in range(4)]

            def emit_rms_pair(a):
                ps = slice(a * 1024, (a + 1) * 1024)
                for i in range(4):
                    xsq = xsqp.tile([128, 1024], F16, tag="xsq", name="xsq")
                    if a == 0:
                        nc.vector.tensor_mul(xsq[:], xt[i][:, ps],
                                             xt[i][:, ps])
                    else:
                        nc.gpsimd.tensor_mul(xsq[:], xt[i][:, ps],
                                             xt[i][:, ps])
                    for half in range(2):
                        hs = slice(half * 512, (half + 1) * 512)
                        nc.tensor.matmul(ss_ab[a][:, hs], ones128[:],
                                         xsq[:, hs], start=(i == 0),
                                         stop=(i == 3),
                                         skip_group_check=True)
                # sn = sqrt(sumsq/512)  ->  sinv = sqrt(512)/||x||
                nc.scalar.activation(sn[:, ps], ss_ab[a][:],
                                     mybir.ActivationFunctionType.Sqrt,
                                     scale=1.0 / DIM)
                nc.vector.reciprocal_approx_fast(sinv[:, ps], sn[:, ps])
                for i in range(4):
                    nc.vector.tensor_mul(xn[i][:, ps], xt[i][:, ps],
                                         sinv[:, ps])

            # ---- Q/K projection + RoPE ----
            # wqk columns: [q h0..h3 | k h0..h3]; m=0: q heads01, m=1: q
            # heads23, m=2: k heads01, m=3: k heads23.  Head d-dims are
            # host-permuted so the RoPE pair-swap is partition p <-> p^16
            # within each 32-partition quadrant (one stream_shuffle).
            qk_dest = []
            for name in ["qr0", "qr1", "kr0", "kr1"]:
                t = persist.tile([128, N], F16, tag=name, name=name)
                qk_dest.append(t)

            def emit_rope_chunk(m, c, cast_eng="vector"):
                ms = slice(m * 128, (m + 1) * 128)
                cs = slice(c * 512, (c + 1) * 512)
                qk = ps_w.tile([128, 512], F32, tag="w", name="qkps")
                for i in range(4):
                    nc.tensor.matmul(qk[:], wqk_t[i][:, ms], xn[i][:, cs],
                                     start=(i == 0), stop=(i == 3))
                qkraw = ropework.tile([128, 512], F16, tag="qkraw", name="qkraw")
                if cast_eng == "scalar":
                    # pre-attention chunks: ACT is idle, use it for the
                    # PSUM evacuation to shorten the serial DVE chain
                    nc.scalar.copy(qkraw[:], qk[:])
                else:
                    nc.vector.tensor_copy(qkraw[:], qk[:])
                rotu = ropework.tile([128, 512], F16, tag="rotu", name="rotu")
                nc.vector.stream_shuffle(rotu[:].bitcast(mybir.dt.int32),
                                         qkraw[:].bitcast(mybir.dt.int32),
                                         SWAP16)
                tmpc = ropework.tile([128, 512], F16, tag="tmpc", name="tmpc")
                nc.vector.tensor_mul(tmpc[:], qkraw[:], cos_t[:, cs])
                rots = ropework.tile([128, 512], F16, tag="rots", name="rots")
                nc.vector.tensor_mul(rots[:], rotu[:], sin_t[:, cs])
                nc.vector.tensor_add(qk_dest[m][:, cs], tmpc[:], rots[:])

            # ---- V projection (token-major), via filler units ----
            def emit_v(tt):
                nc.gpsimd.memset(v_sb[tt][:], 1.0)
                vp = ps_w.tile([128, 512], F32, tag="w", name="vps")
                ts = slice(tt * 128, (tt + 1) * 128)
                for i in range(4):
                    nc.tensor.matmul(vp[:, 0:256], xn[i][:, ts], wv_t[i][:],
                                     start=(i == 0), stop=(i == 3))
                # cols 65h..65h+63 hold head h's V; col 65h+64 stays 1.0
                # (softmax denominator lands in o_ps row 64).
                dst = v_sb[tt][:].rearrange("p (h c) -> p h c", h=4)[:, :, 0:64]
                nc.vector.tensor_copy(dst, vp[:, 0:256].rearrange(
                    "p (h c) -> p h c", h=4))

            # ---- attention for one (query-half, head) ----
            # Software-pipelined: attn@V trails exp by 2 k-tiles, and the
            # final two attn@V tiles + normalize of head h are carried into
            # head h+1's loop so the ACT exp stream never waits.
            outd = [[None, None], [None, None]]  # [hp][qh] -> [128,1024] f16

            def emit_attention(qh, h, fillers, carry, last=False):
                hp, j = h // 2, h % 2
                qr, kr = qk_dest[hp], qk_dest[2 + hp]
                js = slice(j * 64, (j + 1) * 64)
                o_ps = ps_o.tile([65, 1024], F32, tag="o", name="o")
                lag = 1 if last else 2
                es = []
                for kt in range(16):
                    if fillers:
                        fillers.pop(0)()
                    ks = slice(kt * 128, (kt + 1) * 128)
                    s_ps = ps_s.tile([128, 1024], F32, tag="s", name="sc")
                    for sub in range(2):
                        qs = slice(qh * 1024 + sub * 512,
                                   qh * 1024 + (sub + 1) * 512)
                        nc.tensor.matmul(s_ps[:, sub * 512:(sub + 1) * 512],
                                         kr[js, ks], qr[js, qs],
                                         start=True, stop=True)
                    e = exps.tile([128, 1024], F16, tag="e", name="e")
                    nc.scalar.activation(e[:], s_ps[:],
                                         mybir.ActivationFunctionType.Exp,
                                         scale=SCALE)
                    if debug and qh == 0 and h == 0 and kt == 0:
                        nc.sync.dma_start(dbg_e, e[:])
                    es.append(e)
                    if kt == 0 and carry:
                        carry[0]()
                    elif kt == 1 and carry:
                        carry[1]()
                    if kt >= lag:
                        _av(o_ps, h, kt - lag, es[kt - lag], kt == lag, False)
                if last:
                    # tail: finish attn@V and normalize straight from PSUM
                    _av(o_ps, h, 15, es[15], False, True)
                    dcopy = normp.tile([1, 1024], F32, tag="dcopy",
                                       name="dcopy")
                    nc.vector.tensor_copy(dcopy[:], o_ps[64:65, :])
                    if outd[hp][qh] is None:
                        outd[hp][qh] = persist.tile(
                            [128, 1024], F16, tag=f"od{hp}{qh}",
                            name=f"od{hp}{qh}_l")
                    rrow = normp.tile([1, 1024], F32, tag="rrow", name="rrow")
                    nc.vector.reciprocal_approx_fast(rrow[:], dcopy[:])
                    rfull = normp.tile([64, 1024], F32, tag="rfull",
                                       name="rfull")
                    nc.gpsimd.partition_broadcast(rfull[:], rrow[:])
                    nc.vector.tensor_mul(outd[hp][qh][js, :], o_ps[0:64, :],
                                         rfull[:])
                    return []

                def carry_av():
                    _av(o_ps, h, 14, es[14], False, False)
                    _av(o_ps, h, 15, es[15], False, True)

                def carry_norm():
                    oraw = orawp.tile([64, 1024], F32, tag="oraw", name="oraw")
                    nc.vector.tensor_copy(oraw[:], o_ps[0:64, :])
                    dcopy = normp.tile([1, 1024], F32, tag="dcopy",
                                       name="dcopy")
                    nc.vector.tensor_copy(dcopy[:], o_ps[64:65, :])
                    if debug and qh == 0 and h == 0:
                        nc.sync.dma_start(dbg_oraw[0:64, :], oraw[:])
                        nc.sync.dma_start(dbg_oraw[64:65, :], dcopy[:])
                    if outd[hp][qh] is None:
                        od = persist.tile([128, 1024], F16, tag=f"od{hp}{qh}",
                                          name=f"od{hp}{qh}")
                        outd[hp][qh] = od
                    # gpsimd/custom-DVE ucode mishandles nonzero partition
                    # offsets on HW: extract the denominator row with a
                    # plain copy so recip/broadcast see offset-0 APs.
                    rrow = normp.tile([1, 1024], F32, tag="rrow", name="rrow")
                    nc.vector.reciprocal_approx_fast(rrow[:], dcopy[:])
                    rfull = normp.tile([64, 1024], F32, tag="rfull",
                                       name="rfull")
                    nc.gpsimd.partition_broadcast(rfull[:], rrow[:])
                    nc.vector.tensor_mul(outd[hp][qh][js, :], oraw[:],
                                         rfull[:])

                return [carry_av, carry_norm]

            def _av(o_ps, h, kt, e, start, stop):
                for sub in range(2):
                    ss_ = slice(sub * 512, (sub + 1) * 512)
                    nc.tensor.matmul(o_ps[:, ss_],
                                     v_sb[kt][:, 65 * h:65 * h + 65],
                                     e[:, ss_], start=start, stop=stop,
                                     skip_group_check=True)

            # ---- output projection for one query-half (partial w_out) ----
            def emit_proj_chunk(qh, om, sub, copy_eng="vector", dma_each=False):
                oms = slice(om * 128, (om + 1) * 128)
                ss_ = slice(sub * 512, (sub + 1) * 512)
                yp = ps_w.tile([128, 512], F32, tag="w", name="yp")
                for p in range(2):
                    nc.tensor.matmul(yp[:], wo_t[p][:, oms],
                                     outd[p][qh][:, ss_],
                                     start=(p == 0), stop=(p == 1))
                if sub == 0:
                    _ylast[0] = ysbp.tile([128, 1024], F32, tag="y", name="y")
                ysb = _ylast[0]
                if copy_eng == "vector":
                    nc.vector.tensor_copy(ysb[:, ss_], yp[:])
                else:
                    nc.scalar.copy(ysb[:, ss_], yp[:])
                if dma_each:
                    nc.sync.dma_start(
                        yT[oms, qh * 1024 + sub * 512:
                           qh * 1024 + (sub + 1) * 512], ysb[:, ss_])
                elif sub == 1:
                    nc.sync.dma_start(
                        yT[oms, qh * 1024:(qh + 1) * 1024], ysb[:])

            _ylast = [None]

            # ---- emission order ----
            # Minimal prefix so attention(qh0, h0) starts as early as
            # possible: rms pair 0, the three gating rope chunks (kr0 c0 +
            # qr0 c0/c1), then the rest of the pre-attention work.
            emit_rms_pair(0)
            emit_rope_chunk(2, 0, "scalar")    # kr0 c0
            emit_rope_chunk(0, 0, "scalar")    # qr0 c0
            emit_rope_chunk(0, 1, "scalar")    # qr0 c1
            emit_rms_pair(1)
            for tt in range(6):
                emit_v(tt)
            emit_rope_chunk(2, 1, "scalar")    # kr0 c1..c3
            emit_rope_chunk(2, 2, "scalar")
            emit_rope_chunk(2, 3, "scalar")

            fillers = []
            for tt in range(6, 16):
                fillers.append(lambda tt=tt: emit_v(tt))
            for c in range(4):
                fillers.append(lambda c=c: emit_rope_chunk(3, c))   # kr1
            for c in range(2):
                fillers.append(lambda c=c: emit_rope_chunk(1, c))   # qr1 qh0
            for c in range(2, 4):
                fillers.append(lambda c=c: emit_rope_chunk(0, c))   # qr0 qh1
            for c in range(2, 4):
                fillers.append(lambda c=c: emit_rope_chunk(1, c))   # qr1 qh1

            carry = []
            for h in range(4):
                carry = emit_attention(0, h, fillers, carry)
            if debug:
                nc.sync.dma_start(dbg_qr, qk_dest[0][:])
                nc.sync.dma_start(dbg_kr, qk_dest[2][:])
                nc.sync.dma_start(dbg_v, v_sb[0][:])
                nc.sync.dma_start(dbg_sinv, sinv[:])
            # outproj(qh0) becomes fillers, but only after att(1,0) has
            # consumed the carried normalize of (qh0, h3): delay them into
            # the att(1,1)+ loops to keep the DVE stream deadlock-free.
            carry = emit_attention(1, 0, fillers, carry)
            for om in range(4):
                for sub in range(2):
                    fillers.append(
                        lambda om=om, sub=sub: emit_proj_chunk(0, om, sub))
            for h in range(1, 4):
                carry = emit_attention(1, h, fillers, carry, last=(h == 3))
            assert not fillers and not carry
            if debug:
                nc.sync.dma_start(dbg_outd, outd[0][0][:])
            for om in range(4):
                for sub in range(2):
                    emit_proj_chunk(1, om, sub,
                                    copy_eng="scalar" if om % 2 else "vector")

    nc.compile()
    return nc


# Device row r (within a 64-row head slot) holds head-dim PERM64[r]:
# quadrant-local 16-interleave so the RoPE pair partner is at r^16.
PERM64 = np.array([2 * (16 * (r // 32) + r % 16) + ((r % 32) // 16)
                   for r in range(D)])
_SUB = np.array([(r % 32) // 16 for r in range(D)])
_J = np.array([16 * (r // 32) + r % 16 for r in range(D)])


def rope_tables():
    """cos / sign-folded sin tables in PERM64 row order, two head slots."""
    inv_freq = (1.0 / (ROPE_THETA ** (np.arange(0, D, 2, dtype=np.float32) / D)))
    freqs = np.arange(N, dtype=np.float32)[:, None] * inv_freq[None, :]  # [N,32]
    cos64 = np.cos(freqs[:, _J]).T.astype(np.float32)   # [64, N]
    sin64 = np.sin(freqs[:, _J]).T.astype(np.float32)
    sinF64 = np.where(_SUB[:, None] == 0, -sin64, sin64)
    cos2 = np.concatenate([cos64, cos64], axis=0)        # [128, N]
    sinF2 = np.concatenate([sinF64, sinF64], axis=0)
    return (np.ascontiguousarray(cos2).astype(np.float16),
            np.ascontiguousarray(sinF2).astype(np.float16))


def _permute_heads(w):
    """Permute each head's 64 columns of w [512, 256] to PERM64 order."""
    w = w.reshape(DIM, 4, D)[:, :, PERM64]
    return w.reshape(DIM, 256)


def make_in_maps(x, gamma, w_qkv, w_out):
    cos2, sinF2 = rope_tables()
    wg = (gamma[:, None] * w_qkv).astype(np.float32)  # fold gamma
    in_maps = []
    for c in range(NCORES):
        b, g = c // 2, c % 2
        hs = slice(g * 256, (g + 1) * 256)
        wqk_c = np.concatenate([_permute_heads(wg[:, 0:512][:, hs]),
                                _permute_heads(wg[:, 512:1024][:, hs])],
                               axis=1)
        wv_c = wg[:, 1024:1536][:, hs]
        wo_c = w_out[hs, :]
        in_maps.append({
            "xT": np.ascontiguousarray(x[b].T).astype(np.float16),
            "wqk": np.ascontiguousarray(wqk_c).astype(np.float16),
            "wv": np.ascontiguousarray(wv_c).astype(np.float16),
            "wo": np.ascontiguousarray(wo_c).astype(np.float16),
            "cos2": cos2,
            "sinF2": sinF2,
            "ones": np.ones((128, 128), dtype=np.float16),
        })
    return in_maps


_NC_CACHE = None


def _get_program():
    global _NC_CACHE
    if _NC_CACHE is None:
        _NC_CACHE = build_program()
    return _NC_CACHE


def run_cores(inputs, trace=False):
    """Run the SPMD kernel on 8 cores; returns (full_output, results)."""
    from concourse.bass_utils import run_bass_kernel_spmd

    nc = _get_program()
    in_maps = make_in_maps(inputs["x"], inputs["gamma"],
                           inputs["w_qkv"], inputs["w_out"])
    kwargs = {}
    if trace:
        _install_ntff_hook()
        kwargs = dict(trace=True, trace_cores=list(range(NCORES)))
    res = run_bass_kernel_spmd(nc, in_maps, core_ids=list(range(NCORES)),
                               **kwargs)
    out = np.empty((B, N, DIM), dtype=np.float32)
    for b in range(B):
        yTv = res.results[2 * b]["yT"] + res.results[2 * b + 1]["yT"]
        out[b] = yTv.T
    return out, res


def _install_ntff_hook():
    """Register the axon NTFF profiling hook (missing antenv.axon_hooks)."""
    import sys
    import types

    if "antenv.axon_hooks" in sys.modules:
        return
    try:
        import trn_agent_boot.trn_boot as tb
        import concourse.bass_utils as bu

        mod = types.ModuleType("antenv.axon_hooks")
        hook = tb._ntff_profile_via_ctypes("/opt/axon/libaxon_pjrt.so")
        mod.get_axon_ntff_profile_hook = lambda: hook
        sys.modules["antenv.axon_hooks"] = mod
        bu.upload_artifacts = lambda tmpdir: "local://" + tmpdir
    except Exception:
        pass


def kernel(**inputs):
    out, _ = run_cores(inputs, trace=bool(os.environ.get("KERNEL_TRACE")))
    return out


# revision 38
# speedup vs baseline: 1.0324x; 1.0159x over previous
"""AttentionWithRoPE Trainium2 kernel (8-core SPMD), v2.

Sharding: core c handles batch b = c // 2 and head-group g = c % 2
(heads 4g..4g+3).  Each core computes rmsnorm(x_b), its 4 heads' Q/K/V
projections, RoPE, full-sequence attention, and a partial output
projection (its heads' rows of w_out).  Host sums the two partial
outputs per batch.

v2 changes vs v1:
- fp16 everywhere on the PE (fp32-HIGH mode and its FWL-disable penalty
  are gone); accumulation stays fp32 in PSUM.
- RoPE pair-swap is a single DVE stream_shuffle: head dims are
  host-permuted so swap partners sit +-16 apart inside each 32-partition
  quadrant (stream_shuffle can only permute within quadrants).
- Attention runs one head at a time, software-pipelined: per k-tile the
  PE emits logits, ACT exps them (1024 wide), and the PE retires the
  previous k-tile's attn@V while the exp runs.  ACT (exp) is the
  critical engine; the PE stream is kept dense so the HAM clock stays
  at 2.4 GHz.
- Long-latency work (V projection, later RoPE chunks, output projection
  tiles) is emitted as "fillers" inside attention kt loops so no engine
  idles between phases.
- Softmax denominator comes from a ones column appended to V; the
  normalize (broadcast/recip/mul) is deferred off the critical path via
  an SBUF copy of the raw attention output.
- PSUM budget (8 banks): work pool 2, logits double-buffer 4, attn-out 2.
"""

import os

import numpy as np

import concourse.bass as bass
import concourse.tile as tile
from concourse import bacc, mybir

B, N, DIM = 4, 2048, 512
H, D = 8, 64
ROPE_THETA = 10000.0
NCORES = 8
SCALE = D ** -0.5

F32 = mybir.dt.float32
F16 = mybir.dt.float16

SWAP16 = [(i + 16) % 32 for i in range(32)]


def build_program():
    nc = bacc.Bacc("TRN2", target_bir_lowering=False, debug=False)

    xT = nc.dram_tensor("xT", [DIM, N], F16, kind="ExternalInput").ap()
    wqk = nc.dram_tensor("wqk", [DIM, 512], F16, kind="ExternalInput").ap()
    wv = nc.dram_tensor("wv", [DIM, 256], F16, kind="ExternalInput").ap()
    wo = nc.dram_tensor("wo", [256, DIM], F16, kind="ExternalInput").ap()
    cos2 = nc.dram_tensor("cos2", [128, N], F16, kind="ExternalInput").ap()
    sinF2 = nc.dram_tensor("sinF2", [128, N], F16, kind="ExternalInput").ap()
    ones_d = nc.dram_tensor("ones", [128, 128], F16, kind="ExternalInput").ap()
    yT = nc.dram_tensor("yT", [DIM, N], F32, kind="ExternalOutput").ap()
    debug = bool(os.environ.get("KERNEL_DEBUG"))
    if debug:
        dbg_qr = nc.dram_tensor("dbg_qr", [128, N], F16, kind="ExternalOutput").ap()
        dbg_kr = nc.dram_tensor("dbg_kr", [128, N], F16, kind="ExternalOutput").ap()
        dbg_e = nc.dram_tensor("dbg_e", [128, 1024], F16, kind="ExternalOutput").ap()
        dbg_oraw = nc.dram_tensor("dbg_oraw", [65, 1024], F32, kind="ExternalOutput").ap()
        dbg_outd = nc.dram_tensor("dbg_outd", [128, 1024], F16, kind="ExternalOutput").ap()
        dbg_v = nc.dram_tensor("dbg_v", [128, 260], F16, kind="ExternalOutput").ap()
        dbg_sinv = nc.dram_tensor("dbg_sinv", [128, N], F32, kind="ExternalOutput").ap()

    with tile.TileContext(nc) as tc:
        with tc.tile_pool(name="persist", bufs=1) as persist, \
             tc.tile_pool(name="xsqp", bufs=2) as xsqp, \
             tc.tile_pool(name="ropework", bufs=3) as ropework, \
             tc.tile_pool(name="exps", bufs=8) as exps, \
             tc.tile_pool(name="orawp", bufs=3) as orawp, \
             tc.tile_pool(name="normp", bufs=2) as normp, \
             tc.tile_pool(name="ysbp", bufs=2) as ysbp, \
             tc.tile_pool(name="ps_w", bufs=2, space="PSUM") as ps_w, \
             tc.tile_pool(name="ps_s", bufs=2, space="PSUM") as ps_s, \
             tc.tile_pool(name="ps_o", bufs=1, space="PSUM") as ps_o:

            # ---- input DMAs, ordered so the first 1024 tokens' rmsnorm
            # pipeline starts while the rest stream in ----
            xt = [persist.tile([128, N], F16, tag=f"xt{i}", name=f"xt_{i}",
                               uniquify=False) for i in range(4)]
            for i in range(4):
                nc.sync.dma_start(xt[i][:], xT[i * 128:(i + 1) * 128, :])
            ones128 = persist.tile([128, 128], F16, tag="ones128", name="ones128")
            nc.sync.dma_start(ones128[:], ones_d)
            cos_t = persist.tile([128, N], F16, tag="cos", name="cos")
            nc.sync.dma_start(cos_t[:], cos2)
            sin_t = persist.tile([128, N], F16, tag="sin", name="sin")
            nc.sync.dma_start(sin_t[:], sinF2)
            wqk_t = []
            for i in range(4):
                t = persist.tile([128, 512], F16, tag=f"wqk{i}", name=f"wqk{i}")
                nc.sync.dma_start(t[:], wqk[i * 128:(i + 1) * 128, :])
                wqk_t.append(t)
            wv_t = []
            for i in range(4):
                t = persist.tile([128, 256], F16, tag=f"wv{i}", name=f"wv{i}")
                nc.sync.dma_start(t[:], wv[i * 128:(i + 1) * 128, :])
                wv_t.append(t)
            wo_t = []
            for p in range(2):
                t = persist.tile([128, 512], F16, tag=f"wo{p}", name=f"wo{p}")
                nc.sync.dma_start(t[:], wo[p * 128:(p + 1) * 128, :])
                wo_t.append(t)
            # V tiles: memset to 1.0 (in emit_v); the V-copy overwrites the
            # data columns, leaving the per-head ones column (softmax
            # denominator) intact.
            v_sb = [persist.tile([128, 260], F16, tag=f"v{tt}", name=f"v_{tt}",
                                 uniquify=False) for tt in range(16)]

            # ---- rmsnorm, per 1024-token pair: sumsq via ones-matmul,
            # sqrt, recip, xn.  Pair 0 squares on DVE (critical path to the
            # first exp); pair 1 on the otherwise-idle gpsimd. ----
            ss_ab = [ps_s.tile([128, 1024], F32, tag="s", name=f"ss{a}")
                     for a in range(2)]
            sn = persist.tile([128, N], F32, tag="sn", name="sn")
            sinv = persist.tile([128, N], F32, tag="sinv", name="sinv")
            xn = [persist.tile([128, N], F16, tag=f"xn{i}", name=f"xn_{i}",
                               uniquify=False) for i in range(4)]

            def emit_rms_pair(a):
                ps = slice(a * 1024, (a + 1) * 1024)
                for i in range(4):
                    xsq = xsqp.tile([128, 1024], F16, tag="xsq", name="xsq")
                    if a == 0:
                        nc.vector.tensor_mul(xsq[:], xt[i][:, ps],
                                             xt[i][:, ps])
                    else:
                        nc.gpsimd.tensor_mul(xsq[:], xt[i][:, ps],
                                             xt[i][:, ps])
                    for half in range(2):
                        hs = slice(half * 512, (half + 1) * 512)
                        nc.tensor.matmul(ss_ab[a][:, hs], ones128[:],
                                         xsq[:, hs], start=(i == 0),
                                         stop=(i == 3),
                                         skip_group_check=True)
                # sn = sqrt(sumsq/512)  ->  sinv = sqrt(512)/||x||
                nc.scalar.activation(sn[:, ps], ss_ab[a][:],
                                     mybir.ActivationFunctionType.Sqrt,
                                     scale=1.0 / DIM)
                nc.vector.reciprocal_approx_fast(sinv[:, ps], sn[:, ps])
                for i in range(4):
                    nc.vector.tensor_mul(xn[i][:, ps], xt[i][:, ps],
                                         sinv[:, ps])

            # ---- Q/K projection + RoPE ----
            # wqk columns: [q h0..h3 | k h0..h3]; m=0: q heads01, m=1: q
            # heads23, m=2: k heads01, m=3: k heads23.  Head d-dims are
            # host-permuted so the RoPE pair-swap is partition p <-> p^16
            # within each 32-partition quadrant (one stream_shuffle).
            qk_dest = []
            for name in ["qr0", "qr1", "kr0", "kr1"]:
                t = persist.tile([128, N], F16, tag=name, name=name)
                qk_dest.append(t)

            def emit_rope_chunk(m, c, cast_eng="vector"):
                ms = slice(m * 128, (m + 1) * 128)
                cs = slice(c * 512, (c + 1) * 512)
                qk = ps_w.tile([128, 512], F32, tag="w", name="qkps")
                for i in range(4):
                    nc.tensor.matmul(qk[:], wqk_t[i][:, ms], xn[i][:, cs],
                                     start=(i == 0), stop=(i == 3))
                qkraw = ropework.tile([128, 512], F16, tag="qkraw", name="qkraw")
                if cast_eng == "scalar":
                    # pre-attention chunks: ACT is idle, use it for the
                    # PSUM evacuation to shorten the serial DVE chain
                    nc.scalar.copy(qkraw[:], qk[:])
                else:
                    nc.vector.tensor_copy(qkraw[:], qk[:])
                rotu = ropework.tile([128, 512], F16, tag="rotu", name="rotu")
                nc.vector.stream_shuffle(rotu[:].bitcast(mybir.dt.int32),
                                         qkraw[:].bitcast(mybir.dt.int32),
                                         SWAP16)
                tmpc = ropework.tile([128, 512], F16, tag="tmpc", name="tmpc")
                nc.vector.tensor_mul(tmpc[:], qkraw[:], cos_t[:, cs])
                rots = ropework.tile([128, 512], F16, tag="rots", name="rots")
                nc.vector.tensor_mul(rots[:], rotu[:], sin_t[:, cs])
                nc.vector.tensor_add(qk_dest[m][:, cs], tmpc[:], rots[:])

            # ---- V projection (token-major), via filler units ----
            def emit_v(tt):
                nc.gpsimd.memset(v_sb[tt][:], 1.0)
                vp = ps_w.tile([128, 512], F32, tag="w", name="vps")
                ts = slice(tt * 128, (tt + 1) * 128)
                for i in range(4):
                    nc.tensor.matmul(vp[:, 0:256], xn[i][:, ts], wv_t[i][:],
                                     start=(i == 0), stop=(i == 3))
                # cols 65h..65h+63 hold head h's V; col 65h+64 stays 1.0
                # (softmax denominator lands in o_ps row 64).
                dst = v_sb[tt][:].rearrange("p (h c) -> p h c", h=4)[:, :, 0:64]
                nc.vector.tensor_copy(dst, vp[:, 0:256].rearrange(
                    "p (h c) -> p h c", h=4))

            # ---- attention for one (query-half, head) ----
            # Software-pipelined: attn@V trails exp by 2 k-tiles, and the
            # final two attn@V tiles + normalize of head h are carried into
            # head h+1's loop so the ACT exp stream never waits.
            outd = [[None, None], [None, None]]  # [hp][qh] -> [128,1024] f16

            def emit_attention(qh, h, fillers, carry, last=False):
                hp, j = h // 2, h % 2
                qr, kr = qk_dest[hp], qk_dest[2 + hp]
                js = slice(j * 64, (j + 1) * 64)
                o_ps = ps_o.tile([65, 1024], F32, tag="o", name="o")
                lag = 1 if last else 2
                es = []
                for kt in range(16):
                    if fillers:
                        fillers.pop(0)()
                    ks = slice(kt * 128, (kt + 1) * 128)
                    s_ps = ps_s.tile([128, 1024], F32, tag="s", name="sc")
                    for sub in range(2):
                        qs = slice(qh * 1024 + sub * 512,
                                   qh * 1024 + (sub + 1) * 512)
                        nc.tensor.matmul(s_ps[:, sub * 512:(sub + 1) * 512],
                                         kr[js, ks], qr[js, qs],
                                         start=True, stop=True)
                    e = exps.tile([128, 1024], F16, tag="e", name="e")
                    nc.scalar.activation(e[:], s_ps[:],
                                         mybir.ActivationFunctionType.Exp,
                                         scale=SCALE)
                    if debug and qh == 0 and h == 0 and kt == 0:
                        nc.sync.dma_start(dbg_e, e[:])
                    es.append(e)
                    if kt == 0 and carry:
                        carry[0]()
                    elif kt == 1 and carry:
                        carry[1]()
                    if kt >= lag:
                        _av(o_ps, h, kt - lag, es[kt - lag], kt == lag, False)
                if last:
                    # tail: finish attn@V and normalize straight from PSUM
                    _av(o_ps, h, 15, es[15], False, True)
                    dcopy = normp.tile([1, 1024], F32, tag="dcopy",
                                       name="dcopy")
                    nc.vector.tensor_copy(dcopy[:], o_ps[64:65, :])
                    if outd[hp][qh] is None:
                        outd[hp][qh] = persist.tile(
                            [128, 1024], F16, tag=f"od{hp}{qh}",
                            name=f"od{hp}{qh}_l")
                    rrow = normp.tile([1, 1024], F32, tag="rrow", name="rrow")
                    nc.vector.reciprocal_approx_fast(rrow[:], dcopy[:])
                    rfull = normp.tile([64, 1024], F32, tag="rfull",
                                       name="rfull")
                    nc.gpsimd.partition_broadcast(rfull[:], rrow[:])
                    nc.vector.tensor_mul(outd[hp][qh][js, :], o_ps[0:64, :],
                                         rfull[:])
                    return []

                def carry_av():
                    _av(o_ps, h, 14, es[14], False, False)
                    _av(o_ps, h, 15, es[15], False, True)

                def carry_norm():
                    oraw = orawp.tile([64, 1024], F32, tag="oraw", name="oraw")
                    nc.vector.tensor_copy(oraw[:], o_ps[0:64, :])
                    dcopy = normp.tile([1, 1024], F32, tag="dcopy",
                                       name="dcopy")
                    nc.vector.tensor_copy(dcopy[:], o_ps[64:65, :])
                    if debug and qh == 0 and h == 0:
                        nc.sync.dma_start(dbg_oraw[0:64, :], oraw[:])
                        nc.sync.dma_start(dbg_oraw[64:65, :], dcopy[:])
                    if outd[hp][qh] is None:
                        od = persist.tile([128, 1024], F16, tag=f"od{hp}{qh}",
                                          name=f"od{hp}{qh}")
                        outd[hp][qh] = od
                    # gpsimd/custom-DVE ucode mishandles nonzero partition
                    # offsets on HW: extract the denominator row with a
                    # plain copy so recip/broadcast see offset-0 APs.
                    rrow = normp.tile([1, 1024], F32, tag="rrow", name="rrow")
                    nc.vector.reciprocal_approx_fast(rrow[:], dcopy[:])
                    rfull = normp.tile([64, 1024], F32, tag="rfull",
                                       name="rfull")
                    nc.gpsimd.partition_broadcast(rfull[:], rrow[:])
                    nc.vector.tensor_mul(outd[hp][qh][js, :], oraw[:],
                                         rfull[:])

                return [carry_av, carry_norm]

            def _av(o_ps, h, kt, e, start, stop):
                for sub in range(2):
                    ss_ = slice(sub * 512, (sub + 1) * 512)
                    nc.tensor.matmul(o_ps[:, ss_],
                                     v_sb[kt][:, 65 * h:65 * h + 65],
                                     e[:, ss_], start=start, stop=stop,
                                     skip_group_check=True)

            # ---- output projection for one query-half (partial w_out) ----
            def emit_proj_chunk(qh, om, sub, copy_eng="vector", dma_each=False):
                oms = slice(om * 128, (om + 1) * 128)
                ss_ = slice(sub * 512, (sub + 1) * 512)
                yp = ps_w.tile([128, 512], F32, tag="w", name="yp")
                for p in range(2):
                    nc.tensor.matmul(yp[:], wo_t[p][:, oms],
                                     outd[p][qh][:, ss_],
                                     start=(p == 0), stop=(p == 1))
                if sub == 0:
                    _ylast[0] = ysbp.tile([128, 1024], F32, tag="y", name="y")
                ysb = _ylast[0]
                if copy_eng == "vector":
                    nc.vector.tensor_copy(ysb[:, ss_], yp[:])
                else:
                    nc.scalar.copy(ysb[:, ss_], yp[:])
                if dma_each:
                    nc.sync.dma_start(
                        yT[oms, qh * 1024 + sub * 512:
                           qh * 1024 + (sub + 1) * 512], ysb[:, ss_])
                elif sub == 1:
                    nc.sync.dma_start(
                        yT[oms, qh * 1024:(qh + 1) * 1024], ysb[:])

            _ylast = [None]

            # ---- emission order ----
            # Minimal prefix so attention(qh0, h0) starts as early as
            # possible: rms pair 0, the three gating rope chunks (kr0 c0 +
            # qr0 c0/c1), then the rest of the pre-attention work.
            emit_rms_pair(0)
            emit_rope_chunk(2, 0, "scalar")    # kr0 c0
            emit_rope_chunk(0, 0, "scalar")    # qr0 c0
            emit_rope_chunk(0, 1, "scalar")    # qr0 c1
            emit_rms_pair(1)
            for tt in range(6):
                emit_v(tt)
            emit_rope_chunk(2, 1)              # kr0 c1..c3
            emit_rope_chunk(2, 2)
            emit_rope_chunk(2, 3)

            fillers = []
            for tt in range(6, 16):
                fillers.append(lambda tt=tt: emit_v(tt))
            for c in range(4):
                fillers.append(lambda c=c: emit_rope_chunk(3, c))   # kr1
            for c in range(2):
                fillers.append(lambda c=c: emit_rope_chunk(1, c))   # qr1 qh0
            for c in range(2, 4):
                fillers.append(lambda c=c: emit_rope_chunk(0, c))   # qr0 qh1
            for c in range(2, 4):
                fillers.append(lambda c=c: emit_rope_chunk(1, c))   # qr1 qh1

            carry = []
            for h in range(4):
                carry = emit_attention(0, h, fillers, carry)
            if debug:
                nc.sync.dma_start(dbg_qr, qk_dest[0][:])
                nc.sync.dma_start(dbg_kr, qk_dest[2][:])
                nc.sync.dma_start(dbg_v, v_sb[0][:])
                nc.sync.dma_start(dbg_sinv, sinv[:])
            # outproj(qh0) becomes fillers, but only after att(1,0) has
            # consumed the carried normalize of (qh0, h3): delay them into
            # the att(1,1)+ loops to keep the DVE stream deadlock-free.
            carry = emit_attention(1, 0, fillers, carry)
            for om in range(4):
                for sub in range(2):
                    fillers.append(
                        lambda om=om, sub=sub: emit_proj_chunk(0, om, sub))
            for h in range(1, 4):
                carry = emit_attention(1, h, fillers, carry, last=(h == 3))
            assert not fillers and not carry
            if debug:
                nc.sync.dma_start(dbg_outd, outd[0][0][:])
            for om in range(4):
                for sub in range(2):
                    emit_proj_chunk(1, om, sub,
                                    copy_eng="scalar" if om % 2 else "vector")

    nc.compile()
    return nc


# Device row r (within a 64-row head slot) holds head-dim PERM64[r]:
# quadrant-local 16-interleave so the RoPE pair partner is at r^16.
PERM64 = np.array([2 * (16 * (r // 32) + r % 16) + ((r % 32) // 16)
                   for r in range(D)])
_SUB = np.array([(r % 32) // 16 for r in range(D)])
_J = np.array([16 * (r // 32) + r % 16 for r in range(D)])


def rope_tables():
    """cos / sign-folded sin tables in PERM64 row order, two head slots."""
    inv_freq = (1.0 / (ROPE_THETA ** (np.arange(0, D, 2, dtype=np.float32) / D)))
    freqs = np.arange(N, dtype=np.float32)[:, None] * inv_freq[None, :]  # [N,32]
    cos64 = np.cos(freqs[:, _J]).T.astype(np.float32)   # [64, N]
    sin64 = np.sin(freqs[:, _J]).T.astype(np.float32)
    sinF64 = np.where(_SUB[:, None] == 0, -sin64, sin64)
    cos2 = np.concatenate([cos64, cos64], axis=0)        # [128, N]
    sinF2 = np.concatenate([sinF64, sinF64], axis=0)
    return (np.ascontiguousarray(cos2).astype(np.float16),
            np.ascontiguousarray(sinF2).astype(np.float16))


def _permute_heads(w):
    """Permute each head's 64 columns of w [512, 256] to PERM64 order."""
    w = w.reshape(DIM, 4, D)[:, :, PERM64]
    return w.reshape(DIM, 256)


def make_in_maps(x, gamma, w_qkv, w_out):
    cos2, sinF2 = rope_tables()
    wg = (gamma[:, None] * w_qkv).astype(np.float32)  # fold gamma
    in_maps = []
    for c in range(NCORES):
        b, g = c // 2, c % 2
        hs = slice(g * 256, (g + 1) * 256)
        wqk_c = np.concatenate([_permute_heads(wg[:, 0:512][:, hs]),
                                _permute_heads(wg[:, 512:1024][:, hs])],
                               axis=1)
        wv_c = wg[:, 1024:1536][:, hs]
        wo_c = w_out[hs, :]
        in_maps.append({
            "xT": np.ascontiguousarray(x[b].T).astype(np.float16),
            "wqk": np.ascontiguousarray(wqk_c).astype(np.float16),
            "wv": np.ascontiguousarray(wv_c).astype(np.float16),
            "wo": np.ascontiguousarray(wo_c).astype(np.float16),
            "cos2": cos2,
            "sinF2": sinF2,
            "ones": np.ones((128, 128), dtype=np.float16),
        })
    return in_maps


_NC_CACHE = None


def _get_program():
    global _NC_CACHE
    if _NC_CACHE is None:
        _NC_CACHE = build_program()
    return _NC_CACHE


def run_cores(inputs, trace=False):
    """Run the SPMD kernel on 8 cores; returns (full_output, results)."""
    from concourse.bass_utils import run_bass_kernel_spmd

    nc = _get_program()
    in_maps = make_in_maps(inputs["x"], inputs["gamma"],
                           inputs["w_qkv"], inputs["w_out"])
    kwargs = {}
    if trace:
        _install_ntff_hook()
        kwargs = dict(trace=True, trace_cores=list(range(NCORES)))
    res = run_bass_kernel_spmd(nc, in_maps, core_ids=list(range(NCORES)),
                               **kwargs)
    out = np.empty((B, N, DIM), dtype=np.float32)
    for b in range(B):
        yTv = res.results[2 * b]["yT"] + res.results[2 * b + 1]["yT"]
        out[b] = yTv.T
    return out, res


def _install_ntff_hook():
    """Register the axon NTFF profiling hook (missing antenv.axon_hooks)."""
    import sys
    import types

    if "antenv.axon_hooks" in sys.modules:
        return
    try:
        import trn_agent_boot.trn_boot as tb
        import concourse.bass_utils as bu

        mod = types.ModuleType("antenv.axon_hooks")
        hook = tb._ntff_profile_via_ctypes("/opt/axon/libaxon_pjrt.so")
        mod.get_axon_ntff_profile_hook = lambda: hook
        sys.modules["antenv.axon_hooks"] = mod
        bu.upload_artifacts = lambda tmpdir: "local://" + tmpdir
    except Exception:
        pass


def kernel(**inputs):
    out, _ = run_cores(inputs, trace=bool(os.environ.get("KERNEL_TRACE")))
    return out


# revision 43
# speedup vs baseline: 1.0345x; 1.0020x over previous
"""AttentionWithRoPE Trainium2 kernel (8-core SPMD), v2.

Sharding: core c handles batch b = c // 2 and head-group g = c % 2
(heads 4g..4g+3).  Each core computes rmsnorm(x_b), its 4 heads' Q/K/V
projections, RoPE, full-sequence attention, and a partial output
projection (its heads' rows of w_out).  Host sums the two partial
outputs per batch.

v2 changes vs v1:
- fp16 everywhere on the PE (fp32-HIGH mode and its FWL-disable penalty
  are gone); accumulation stays fp32 in PSUM.
- RoPE pair-swap is a single DVE stream_shuffle: head dims are
  host-permuted so swap partners sit +-16 apart inside each 32-partition
  quadrant (stream_shuffle can only permute within quadrants).
- Attention runs one head at a time, software-pipelined: per k-tile the
  PE emits logits, ACT exps them (1024 wide), and the PE retires the
  previous k-tile's attn@V while the exp runs.  ACT (exp) is the
  critical engine; the PE stream is kept dense so the HAM clock stays
  at 2.4 GHz.
- Long-latency work (V projection, later RoPE chunks, output projection
  tiles) is emitted as "fillers" inside attention kt loops so no engine
  idles between phases.
- Softmax denominator comes from a ones column appended to V; the
  normalize (broadcast/recip/mul) is deferred off the critical path via
  an SBUF copy of the raw attention output.
- PSUM budget (8 banks): work pool 2, logits double-buffer 4, attn-out 2.
"""

import os

import numpy as np

import concourse.bass as bass
import concourse.tile as tile
from concourse import bacc, mybir

B, N, DIM = 4, 2048, 512
H, D = 8, 64
ROPE_THETA = 10000.0
NCORES = 8
SCALE = D ** -0.5

F32 = mybir.dt.float32
F16 = mybir.dt.float16

SWAP16 = [(i + 16) % 32 for i in range(32)]

# Skip the redundant LDWEIGHTS on the second matmul of same-stationary
# pairs (logits q-halves, attn@V q-halves).
NOLDW = os.environ.get("KERNEL_NOLDW", "0") == "1"


def build_program():
    nc = bacc.Bacc("TRN2", target_bir_lowering=False, debug=False)

    xT = nc.dram_tensor("xT", [DIM, N], F16, kind="ExternalInput").ap()
    wqk = nc.dram_tensor("wqk", [DIM, 512], F16, kind="ExternalInput").ap()
    wv = nc.dram_tensor("wv", [DIM, 256], F16, kind="ExternalInput").ap()
    wo = nc.dram_tensor("wo", [256, DIM], F16, kind="ExternalInput").ap()
    cos2 = nc.dram_tensor("cos2", [128, N], F16, kind="ExternalInput").ap()
    sinF2 = nc.dram_tensor("sinF2", [128, N], F16, kind="ExternalInput").ap()
    ones_d = nc.dram_tensor("ones", [128, 128], F16, kind="ExternalInput").ap()
    yT = nc.dram_tensor("yT", [DIM, N], F32, kind="ExternalOutput").ap()
    debug = bool(os.environ.get("KERNEL_DEBUG"))
    if debug:
        dbg_qr = nc.dram_tensor("dbg_qr", [128, N], F16, kind="ExternalOutput").ap()
        dbg_kr = nc.dram_tensor("dbg_kr", [128, N], F16, kind="ExternalOutput").ap()
        dbg_e = nc.dram_tensor("dbg_e", [128, 1024], F16, kind="ExternalOutput").ap()
        dbg_oraw = nc.dram_tensor("dbg_oraw", [65, 1024], F32, kind="ExternalOutput").ap()
        dbg_outd = nc.dram_tensor("dbg_outd", [128, 1024], F16, kind="ExternalOutput").ap()
        dbg_v = nc.dram_tensor("dbg_v", [128, 260], F16, kind="ExternalOutput").ap()
        dbg_sinv = nc.dram_tensor("dbg_sinv", [128, N], F32, kind="ExternalOutput").ap()

    with tile.TileContext(nc) as tc:
        with tc.tile_pool(name="persist", bufs=1) as persist, \
             tc.tile_pool(name="xsqp", bufs=2) as xsqp, \
             tc.tile_pool(name="ropework", bufs=3) as ropework, \
             tc.tile_pool(name="exps", bufs=8) as exps, \
             tc.tile_pool(name="orawp", bufs=3) as orawp, \
             tc.tile_pool(name="normp", bufs=2) as normp, \
             tc.tile_pool(name="ysbp", bufs=2) as ysbp, \
             tc.tile_pool(name="ps_w", bufs=2, space="PSUM") as ps_w, \
             tc.tile_pool(name="ps_s", bufs=2, space="PSUM") as ps_s, \
             tc.tile_pool(name="ps_o", bufs=1, space="PSUM") as ps_o:

            # ---- input DMAs, ordered so the first 1024 tokens' rmsnorm
            # pipeline starts while the rest stream in ----
            xt = [persist.tile([128, N], F16, tag=f"xt{i}", name=f"xt_{i}",
                               uniquify=False) for i in range(4)]
            for i in range(4):
                nc.sync.dma_start(xt[i][:], xT[i * 128:(i + 1) * 128, :])
            ones128 = persist.tile([128, 128], F16, tag="ones128", name="ones128")
            nc.sync.dma_start(ones128[:], ones_d)
            cos_t = persist.tile([128, N], F16, tag="cos", name="cos")
            nc.sync.dma_start(cos_t[:], cos2)
            sin_t = persist.tile([128, N], F16, tag="sin", name="sin")
            nc.sync.dma_start(sin_t[:], sinF2)
            wqk_t = []
            for i in range(4):
                t = persist.tile([128, 512], F16, tag=f"wqk{i}", name=f"wqk{i}")
                nc.sync.dma_start(t[:], wqk[i * 128:(i + 1) * 128, :])
                wqk_t.append(t)
            wv_t = []
            for i in range(4):
                t = persist.tile([128, 256], F16, tag=f"wv{i}", name=f"wv{i}")
                nc.sync.dma_start(t[:], wv[i * 128:(i + 1) * 128, :])
                wv_t.append(t)
            wo_t = []
            for p in range(2):
                t = persist.tile([128, 512], F16, tag=f"wo{p}", name=f"wo{p}")
                nc.sync.dma_start(t[:], wo[p * 128:(p + 1) * 128, :])
                wo_t.append(t)
            # V tiles: memset to 1.0 (in emit_v); the V-copy overwrites the
            # data columns, leaving the per-head ones column (softmax
            # denominator) intact.
            v_sb = [persist.tile([128, 260], F16, tag=f"v{tt}", name=f"v_{tt}",
                                 uniquify=False) for tt in range(16)]

            # ---- rmsnorm, per 1024-token pair: sumsq via ones-matmul,
            # sqrt, recip, xn.  Pair 0 squares on DVE (critical path to the
            # first exp); pair 1 on the otherwise-idle gpsimd. ----
            ss_ab = [ps_s.tile([128, 1024], F32, tag="s", name=f"ss{a}")
                     for a in range(2)]
            sn = persist.tile([128, N], F32, tag="sn", name="sn")
            sinv = persist.tile([128, N], F32, tag="sinv", name="sinv")
            xn = [persist.tile([128, N], F16, tag=f"xn{i}", name=f"xn_{i}",
                               uniquify=False) for i in range(4)]

            def emit_rms_pair(a):
                ps = slice(a * 1024, (a + 1) * 1024)
                for i in range(4):
                    xsq = xsqp.tile([128, 1024], F16, tag="xsq", name="xsq")
                    if a == 0:
                        nc.vector.tensor_mul(xsq[:], xt[i][:, ps],
                                             xt[i][:, ps])
                    else:
                        nc.gpsimd.tensor_mul(xsq[:], xt[i][:, ps],
                                             xt[i][:, ps])
                    for half in range(2):
                        hs = slice(half * 512, (half + 1) * 512)
                        nc.tensor.matmul(ss_ab[a][:, hs], ones128[:],
                                         xsq[:, hs], start=(i == 0),
                                         stop=(i == 3),
                                         skip_group_check=True)
                # sn = sqrt(sumsq/512)  ->  sinv = sqrt(512)/||x||
                nc.scalar.activation(sn[:, ps], ss_ab[a][:],
                                     mybir.ActivationFunctionType.Sqrt,
                                     scale=1.0 / DIM)
                nc.vector.reciprocal_approx_fast(sinv[:, ps], sn[:, ps])
                for i in range(4):
                    nc.vector.tensor_mul(xn[i][:, ps], xt[i][:, ps],
                                         sinv[:, ps])

            # ---- Q/K projection + RoPE ----
            # wqk columns: [q h0..h3 | k h0..h3]; m=0: q heads01, m=1: q
            # heads23, m=2: k heads01, m=3: k heads23.  Head d-dims are
            # host-permuted so the RoPE pair-swap is partition p <-> p^16
            # within each 32-partition quadrant (one stream_shuffle).
            qk_dest = []
            for name in ["qr0", "qr1", "kr0", "kr1"]:
                t = persist.tile([128, N], F16, tag=name, name=name)
                qk_dest.append(t)

            def emit_rope_chunk(m, c, cast_eng="vector"):
                ms = slice(m * 128, (m + 1) * 128)
                cs = slice(c * 512, (c + 1) * 512)
                qk = ps_w.tile([128, 512], F32, tag="w", name="qkps")
                for i in range(4):
                    nc.tensor.matmul(qk[:], wqk_t[i][:, ms], xn[i][:, cs],
                                     start=(i == 0), stop=(i == 3))
                qkraw = ropework.tile([128, 512], F16, tag="qkraw", name="qkraw")
                if cast_eng == "scalar":
                    # pre-attention chunks: ACT is idle, use it for the
                    # PSUM evacuation to shorten the serial DVE chain
                    nc.scalar.copy(qkraw[:], qk[:])
                else:
                    nc.vector.tensor_copy(qkraw[:], qk[:])
                rotu = ropework.tile([128, 512], F16, tag="rotu", name="rotu")
                nc.vector.stream_shuffle(rotu[:].bitcast(mybir.dt.int32),
                                         qkraw[:].bitcast(mybir.dt.int32),
                                         SWAP16)
                tmpc = ropework.tile([128, 512], F16, tag="tmpc", name="tmpc")
                nc.vector.tensor_mul(tmpc[:], qkraw[:], cos_t[:, cs])
                rots = ropework.tile([128, 512], F16, tag="rots", name="rots")
                nc.vector.tensor_mul(rots[:], rotu[:], sin_t[:, cs])
                nc.vector.tensor_add(qk_dest[m][:, cs], tmpc[:], rots[:])

            # ---- V projection (token-major), via filler units ----
            def emit_v(tt):
                nc.gpsimd.memset(v_sb[tt][:], 1.0)
                vp = ps_w.tile([128, 512], F32, tag="w", name="vps")
                ts = slice(tt * 128, (tt + 1) * 128)
                for i in range(4):
                    nc.tensor.matmul(vp[:, 0:256], xn[i][:, ts], wv_t[i][:],
                                     start=(i == 0), stop=(i == 3))
                # cols 65h..65h+63 hold head h's V; col 65h+64 stays 1.0
                # (softmax denominator lands in o_ps row 64).
                dst = v_sb[tt][:].rearrange("p (h c) -> p h c", h=4)[:, :, 0:64]
                nc.vector.tensor_copy(dst, vp[:, 0:256].rearrange(
                    "p (h c) -> p h c", h=4))

            # ---- attention for one (query-half, head) ----
            # Software-pipelined: attn@V trails exp by 2 k-tiles, and the
            # final two attn@V tiles + normalize of head h are carried into
            # head h+1's loop so the ACT exp stream never waits.
            outd = [[None, None], [None, None]]  # [hp][qh] -> [128,1024] f16

            def emit_attention(qh, h, fillers, carry, last=False):
                hp, j = h // 2, h % 2
                qr, kr = qk_dest[hp], qk_dest[2 + hp]
                js = slice(j * 64, (j + 1) * 64)
                o_ps = ps_o.tile([65, 1024], F32, tag="o", name="o")
                lag = 1 if last else 2
                es = []
                for kt in range(16):
                    if fillers:
                        fillers.pop(0)()
                    ks = slice(kt * 128, (kt + 1) * 128)
                    s_ps = ps_s.tile([128, 1024], F32, tag="s", name="sc")
                    for sub in range(2):
                        qs = slice(qh * 1024 + sub * 512,
                                   qh * 1024 + (sub + 1) * 512)
                        mm = nc.tensor.matmul(
                            s_ps[:, sub * 512:(sub + 1) * 512],
                            kr[js, ks], qr[js, qs], start=True, stop=True)
                        if NOLDW and sub == 1:
                            mm.ins.ldweights = False
                    e = exps.tile([128, 1024], F16, tag="e", name="e")
                    nc.scalar.activation(e[:], s_ps[:],
                                         mybir.ActivationFunctionType.Exp,
                                         scale=SCALE)
                    if debug and qh == 0 and h == 0 and kt == 0:
                        nc.sync.dma_start(dbg_e, e[:])
                    es.append(e)
                    if kt == 0 and carry:
                        carry[0]()
                    elif kt == 1 and carry:
                        carry[1]()
                    if kt >= lag:
                        _av(o_ps, h, kt - lag, es[kt - lag], kt == lag, False)
                if last:
                    # tail: finish attn@V and normalize straight from PSUM
                    _av(o_ps, h, 15, es[15], False, True)
                    dcopy = normp.tile([1, 1024], F32, tag="dcopy",
                                       name="dcopy")
                    nc.vector.tensor_copy(dcopy[:], o_ps[64:65, :])
                    if outd[hp][qh] is None:
                        outd[hp][qh] = persist.tile(
                            [128, 1024], F16, tag=f"od{hp}{qh}",
                            name=f"od{hp}{qh}_l")
                    rrow = normp.tile([1, 1024], F32, tag="rrow", name="rrow")
                    nc.vector.reciprocal_approx_fast(rrow[:], dcopy[:])
                    rfull = normp.tile([64, 1024], F32, tag="rfull",
                                       name="rfull")
                    nc.gpsimd.partition_broadcast(rfull[:], rrow[:])
                    nc.vector.tensor_mul(outd[hp][qh][js, :], o_ps[0:64, :],
                                         rfull[:])
                    return []

                def carry_av():
                    _av(o_ps, h, 14, es[14], False, False)
                    _av(o_ps, h, 15, es[15], False, True)

                def carry_norm():
                    oraw = orawp.tile([64, 1024], F32, tag="oraw", name="oraw")
                    nc.vector.tensor_copy(oraw[:], o_ps[0:64, :])
                    dcopy = normp.tile([1, 1024], F32, tag="dcopy",
                                       name="dcopy")
                    nc.vector.tensor_copy(dcopy[:], o_ps[64:65, :])
                    if debug and qh == 0 and h == 0:
                        nc.sync.dma_start(dbg_oraw[0:64, :], oraw[:])
                        nc.sync.dma_start(dbg_oraw[64:65, :], dcopy[:])
                    if outd[hp][qh] is None:
                        od = persist.tile([128, 1024], F16, tag=f"od{hp}{qh}",
                                          name=f"od{hp}{qh}")
                        outd[hp][qh] = od
                    # gpsimd/custom-DVE ucode mishandles nonzero partition
                    # offsets on HW: extract the denominator row with a
                    # plain copy so recip/broadcast see offset-0 APs.
                    rrow = normp.tile([1, 1024], F32, tag="rrow", name="rrow")
                    nc.vector.reciprocal_approx_fast(rrow[:], dcopy[:])
                    rfull = normp.tile([64, 1024], F32, tag="rfull",
                                       name="rfull")
                    nc.gpsimd.partition_broadcast(rfull[:], rrow[:])
                    nc.vector.tensor_mul(outd[hp][qh][js, :], oraw[:],
                                         rfull[:])

                return [carry_av, carry_norm]

            def _av(o_ps, h, kt, e, start, stop):
                for sub in range(2):
                    ss_ = slice(sub * 512, (sub + 1) * 512)
                    mm = nc.tensor.matmul(o_ps[:, ss_],
                                          v_sb[kt][:, 65 * h:65 * h + 65],
                                          e[:, ss_], start=start, stop=stop,
                                          skip_group_check=True)
                    if NOLDW and sub == 1:
                        mm.ins.ldweights = False

            # ---- output projection for one query-half (partial w_out) ----
            def emit_proj_chunk(qh, om, sub, copy_eng="vector", dma_each=False):
                oms = slice(om * 128, (om + 1) * 128)
                ss_ = slice(sub * 512, (sub + 1) * 512)
                yp = ps_w.tile([128, 512], F32, tag="w", name="yp")
                for p in range(2):
                    nc.tensor.matmul(yp[:], wo_t[p][:, oms],
                                     outd[p][qh][:, ss_],
                                     start=(p == 0), stop=(p == 1))
                if sub == 0:
                    _ylast[0] = ysbp.tile([128, 1024], F32, tag="y", name="y")
                ysb = _ylast[0]
                if copy_eng == "vector":
                    nc.vector.tensor_copy(ysb[:, ss_], yp[:])
                else:
                    nc.scalar.copy(ysb[:, ss_], yp[:])
                if dma_each:
                    nc.sync.dma_start(
                        yT[oms, qh * 1024 + sub * 512:
                           qh * 1024 + (sub + 1) * 512], ysb[:, ss_])
                elif sub == 1:
                    nc.sync.dma_start(
                        yT[oms, qh * 1024:(qh + 1) * 1024], ysb[:])

            _ylast = [None]

            # ---- emission order ----
            # Minimal prefix so attention(qh0, h0) starts as early as
            # possible: rms pair 0, the three gating rope chunks (kr0 c0 +
            # qr0 c0/c1), then the rest of the pre-attention work.
            emit_rms_pair(0)
            emit_rope_chunk(2, 0, "scalar")    # kr0 c0
            emit_rope_chunk(0, 0, "scalar")    # qr0 c0
            emit_rope_chunk(0, 1, "scalar")    # qr0 c1
            emit_rope_chunk(2, 1)              # kr0 c1 (kt4..7 logits)
            for tt in range(8):                # V tiles on pair-0 tokens
                emit_v(tt)
            emit_rms_pair(1)
            emit_rope_chunk(2, 2)              # kr0 c2/c3 need xn pair 1
            emit_rope_chunk(2, 3)

            fillers = []
            for tt in range(8, 16):
                fillers.append(lambda tt=tt: emit_v(tt))
            for c in range(4):
                fillers.append(lambda c=c: emit_rope_chunk(3, c))   # kr1
            for c in range(2):
                fillers.append(lambda c=c: emit_rope_chunk(1, c))   # qr1 qh0
            for c in range(2, 4):
                fillers.append(lambda c=c: emit_rope_chunk(0, c))   # qr0 qh1
            for c in range(2, 4):
                fillers.append(lambda c=c: emit_rope_chunk(1, c))   # qr1 qh1

            carry = []
            for h in range(4):
                carry = emit_attention(0, h, fillers, carry)
            if debug:
                nc.sync.dma_start(dbg_qr, qk_dest[0][:])
                nc.sync.dma_start(dbg_kr, qk_dest[2][:])
                nc.sync.dma_start(dbg_v, v_sb[0][:])
                nc.sync.dma_start(dbg_sinv, sinv[:])
            # outproj(qh0) becomes fillers, but only after att(1,0) has
            # consumed the carried normalize of (qh0, h3): delay them into
            # the att(1,1)+ loops to keep the DVE stream deadlock-free.
            carry = emit_attention(1, 0, fillers, carry)
            for om in range(4):
                for sub in range(2):
                    fillers.append(
                        lambda om=om, sub=sub: emit_proj_chunk(0, om, sub))
            for h in range(1, 4):
                carry = emit_attention(1, h, fillers, carry, last=(h == 3))
            assert not fillers and not carry
            if debug:
                nc.sync.dma_start(dbg_outd, outd[0][0][:])
            for om in range(4):
                for sub in range(2):
                    emit_proj_chunk(1, om, sub,
                                    copy_eng="scalar" if om % 2 else "vector")

    nc.compile()
    return nc


# Device row r (within a 64-row head slot) holds head-dim PERM64[r]:
# quadrant-local 16-interleave so the RoPE pair partner is at r^16.
PERM64 = np.array([2 * (16 * (r // 32) + r % 16) + ((r % 32) // 16)
                   for r in range(D)])
_SUB = np.array([(r % 32) // 16 for r in range(D)])
_J = np.array([16 * (r // 32) + r % 16 for r in range(D)])


def rope_tables():
    """cos / sign-folded sin tables in PERM64 row order, two head slots."""
    inv_freq = (1.0 / (ROPE_THETA ** (np.arange(0, D, 2, dtype=np.float32) / D)))
    freqs = np.arange(N, dtype=np.float32)[:, None] * inv_freq[None, :]  # [N,32]
    cos64 = np.cos(freqs[:, _J]).T.astype(np.float32)   # [64, N]
    sin64 = np.sin(freqs[:, _J]).T.astype(np.float32)
    sinF64 = np.where(_SUB[:, None] == 0, -sin64, sin64)
    cos2 = np.concatenate([cos64, cos64], axis=0)        # [128, N]
    sinF2 = np.concatenate([sinF64, sinF64], axis=0)
    return (np.ascontiguousarray(cos2).astype(np.float16),
            np.ascontiguousarray(sinF2).astype(np.float16))


def _permute_heads(w):
    """Permute each head's 64 columns of w [512, 256] to PERM64 order."""
    w = w.reshape(DIM, 4, D)[:, :, PERM64]
    return w.reshape(DIM, 256)


def make_in_maps(x, gamma, w_qkv, w_out):
    cos2, sinF2 = rope_tables()
    wg = (gamma[:, None] * w_qkv).astype(np.float32)  # fold gamma
    in_maps = []
    for c in range(NCORES):
        b, g = c // 2, c % 2
        hs = slice(g * 256, (g + 1) * 256)
        wqk_c = np.concatenate([_permute_heads(wg[:, 0:512][:, hs]),
                                _permute_heads(wg[:, 512:1024][:, hs])],
                               axis=1)
        wv_c = wg[:, 1024:1536][:, hs]
        wo_c = w_out[hs, :]
        in_maps.append({
            "xT": np.ascontiguousarray(x[b].T).astype(np.float16),
            "wqk": np.ascontiguousarray(wqk_c).astype(np.float16),
            "wv": np.ascontiguousarray(wv_c).astype(np.float16),
            "wo": np.ascontiguousarray(wo_c).astype(np.float16),
            "cos2": cos2,
            "sinF2": sinF2,
            "ones": np.ones((128, 128), dtype=np.float16),
        })
    return in_maps


_NC_CACHE = None


def _get_program():
    global _NC_CACHE
    if _NC_CACHE is None:
        _NC_CACHE = build_program()
    return _NC_CACHE


def run_cores(inputs, trace=False):
    """Run the SPMD kernel on 8 cores; returns (full_output, results)."""
    from concourse.bass_utils import run_bass_kernel_spmd

    nc = _get_program()
    in_maps = make_in_maps(inputs["x"], inputs["gamma"],
                           inputs["w_qkv"], inputs["w_out"])
    kwargs = {}
    if trace:
        _install_ntff_hook()
        kwargs = dict(trace=True, trace_cores=list(range(NCORES)))
    res = run_bass_kernel_spmd(nc, in_maps, core_ids=list(range(NCORES)),
                               **kwargs)
    out = np.empty((B, N, DIM), dtype=np.float32)
    for b in range(B):
        yTv = res.results[2 * b]["yT"] + res.results[2 * b + 1]["yT"]
        out[b] = yTv.T
    return out, res


def _install_ntff_hook():
    """Register the axon NTFF profiling hook (missing antenv.axon_hooks)."""
    import sys
    import types

    if "antenv.axon_hooks" in sys.modules:
        return
    try:
        import trn_agent_boot.trn_boot as tb
        import concourse.bass_utils as bu

        mod = types.ModuleType("antenv.axon_hooks")
        hook = tb._ntff_profile_via_ctypes("/opt/axon/libaxon_pjrt.so")
        mod.get_axon_ntff_profile_hook = lambda: hook
        sys.modules["antenv.axon_hooks"] = mod
        bu.upload_artifacts = lambda tmpdir: "local://" + tmpdir
    except Exception:
        pass


def kernel(**inputs):
    out, _ = run_cores(inputs, trace=bool(os.environ.get("KERNEL_TRACE")))
    return out
